# revision 7
# baseline (speedup 1.0000x reference)
"""GNN message-passing layer (DAGLayer) on 8 Trainium2 NeuronCores.

Strategy (v3):
  - Shard destination nodes 8 ways (12544 nodes = 98 tiles of 128 per core,
    N padded 100000 -> 100352). Replicate h (as a bf16 gather table) so
    cross-partition edges need no communication.
  - Segment-sum over edges via TensorE matmuls: for each 128-dst tile,
    gather h[src] rows (dma_gather, bf16) for its edges in chunks of 128,
    build one-hot P^T on DVE (single-op is_equal vs iota, bf16),
    accumulate segT = G^T @ P^T in PSUM ([dim, dst] layout -> lhsT reuse).
  - Edge indices are int16 for dma_gather, so h is split into 4 row-banks
    of 25088; each (tile, bank) gets a fixed number of 128-edge chunk
    slots (cap_b, normally 2) so the compiled program is identical on all
    cores (SPMD) -- only the data (indices, one-hot columns) differs.
  - Dense path per tile with host-folded weights, all-bf16 matmuls:
      ps_A = h @ [Ws^T | (Wg1+Wg2@Ws)^T]
           + [1|p] @ [[b_s|b_g+Wg2@b_s]; [b_n|Wg2@b_n]]   (K=2 matmul)
      ps_B = seg @ [Wn^T | (Wg2@Wn)^T]
      Y    = s2 * ps_B + ps_A        (DVE stt; s2 = has_pred/max(cnt,1))
      m, gp = Y[:, :128], Y[:, 128:]
      v = h + sigmoid(gp) * (m - h)
      out = relu((v - mu) * rstd)    [bf16, cast to f32 on host]
  - Per-group two-phase scalar schedule to avoid ACT_TABLE_LOAD thrash:
    phase A (gate) uses {Sigmoid, Square, Copy} = sigmoid table; v is
    buffered per group; phase B (layernorm) uses {Sqrt, Relu} = sqrt
    table; LN statistics are computed batched [128, TG] per group.
  - h / hT / out live SBUF-resident for the whole core, loaded/stored via
    three large DMAs (host permutes rows so the block DMA lands node
    t*128+p on partition p).
"""

import numpy as np

try:
    import ml_dtypes

    BF16 = np.dtype(ml_dtypes.bfloat16)
except ImportError:  # pragma: no cover
    BF16 = None

N = 100000
E = 600000
D = 128
N_CORES = 8
NPC = 12544            # nodes per core
TPC = NPC // 128       # 98 tiles per core
N_PAD = NPC * N_CORES  # 100352
N_BANKS = 4
BANK = N_PAD // N_BANKS  # 25088
TG = 14                # tiles per gather group
N_GROUPS = TPC // TG   # 7
LN_EPS = 1e-5


def _build_schedule(edge_src, edge_dst):
    """Chunk/slot schedule. Returns per-core idx16 + d_cols arrays and the
    (core-independent) slot capacities."""
    edge_src = np.asarray(edge_src, np.int64)
    edge_dst = np.asarray(edge_dst, np.int64)

    counts = np.bincount(edge_dst, minlength=N_PAD)

    tile_g = edge_dst // 128          # global dst tile 0..783
    bank = edge_src // BANK           # 0..3
    key = tile_g * N_BANKS + bank
    order = np.argsort(key, kind="stable")
    src_s = edge_src[order]
    dst_s = edge_dst[order]

    n_tiles_g = N_PAD // 128          # 784
    cnt = np.bincount(key[order], minlength=n_tiles_g * N_BANKS).reshape(
        n_tiles_g, N_BANKS
    )
    starts = np.zeros((n_tiles_g, N_BANKS), np.int64)
    starts.reshape(-1)[1:] = np.cumsum(cnt.reshape(-1))[:-1]

    caps = [max(1, int(np.ceil(cnt[:, b].max() / 128.0))) for b in range(N_BANKS)]
    S_t = sum(caps)                   # slots per tile
    cap_off = np.cumsum([0] + caps)   # slot offset of bank b within a tile

    # per-core arrays
    idx16 = np.zeros((N_CORES, N_GROUPS, N_BANKS), object)
    d_cols = np.full((N_CORES, 128, TPC * S_t), -1.0, np.float32)

    for c in range(N_CORES):
        for g in range(N_GROUPS):
            for b in range(N_BANKS):
                lst = np.zeros(TG * caps[b] * 128, np.int16)
                for ti in range(TG):
                    t_local = g * TG + ti
                    t_glob = c * TPC + t_local
                    n_e = int(cnt[t_glob, b])
                    s0 = int(starts[t_glob, b])
                    srcs = src_s[s0 : s0 + n_e] - b * BANK
                    dsts = dst_s[s0 : s0 + n_e] - t_glob * 128
                    base = ti * caps[b] * 128
                    lst[base : base + n_e] = srcs.astype(np.int16)
                    for j in range(caps[b]):
                        col = t_local * S_t + cap_off[b] + j
                        seg = dsts[j * 128 : (j + 1) * 128]
                        d_cols[c, : len(seg), col] = seg
                idx16[c, g, b] = lst
    return idx16, d_cols, caps, S_t, cap_off, counts


def _wrap_idx16(flat):
    """dma_gather index layout: idx i -> [i % 16, i // 16], replicated to
    all 128 partitions."""
    n = len(flat)
    w = np.zeros((128, n // 16), np.int16)
    w16 = flat.reshape(n // 16, 16).T  # [16, n/16]
    w[:16] = w16
    w[16:] = np.tile(w16, (7, 1))
    return w


def _fold_weights(W_self, b_self, W_neigh, b_neigh, W_gate, b_gate):
    Wg1 = W_gate[:, :D]
    Wg2 = W_gate[:, D:]
    rhs_A = np.concatenate([W_self.T, (Wg1 + Wg2 @ W_self).T], 1)  # [128,256]
    rhs_B = np.concatenate([W_neigh.T, (Wg2 @ W_neigh).T], 1)      # [128,256]
    bias_A = np.concatenate([b_self, b_gate + Wg2 @ b_self])        # [256]
    bias_Bp = np.concatenate([b_neigh, Wg2 @ b_neigh])              # [256]
    bias2 = np.stack([bias_A, bias_Bp])                             # [2,256]
    return (
        np.ascontiguousarray(rhs_A).astype(BF16),
        np.ascontiguousarray(rhs_B).astype(BF16),
        np.ascontiguousarray(bias2).astype(BF16),
    )


def _prep(h, edge_src, edge_dst, W_self, b_self, W_neigh, b_neigh, W_gate,
          b_gate, ln_gamma, ln_beta):
    h = np.asarray(h, np.float32)
    h_pad = np.zeros((N_PAD, D), np.float32)
    h_pad[:N] = h
    h_bf16 = np.ascontiguousarray(h_pad).astype(BF16)

    idx16, d_cols, caps, S_t, cap_off, counts = _build_schedule(
        edge_src, edge_dst)

    p = (counts > 0).astype(np.float32)
    s2 = p / np.maximum(counts, 1).astype(np.float32)

    rhs_A, rhs_B, bias2 = _fold_weights(
        np.asarray(W_self, np.float32), np.asarray(b_self, np.float32),
        np.asarray(W_neigh, np.float32), np.asarray(b_neigh, np.float32),
        np.asarray(W_gate, np.float32), np.asarray(b_gate, np.float32))

    trivial_ln = (np.allclose(ln_gamma, 1.0) and np.allclose(ln_beta, 0.0))

    per_core = []
    for c in range(N_CORES):
        rows = slice(c * NPC, (c + 1) * NPC)
        idx_segs = []
        for g in range(N_GROUPS):
            for b in range(N_BANKS):
                idx_segs.append(_wrap_idx16(idx16[c, g, b]))
        idx_all = np.concatenate(idx_segs, axis=1)  # [128, sum(n/16)]
        hc = h_bf16[rows]  # [NPC, 128] bf16
        # block-permuted rows: DRAM row p*TPC + t holds node t*128+p, so one
        # contiguous DMA lands node t*128+p at SBUF [p, t*128:(t+1)*128]
        h_blk = np.ascontiguousarray(
            hc.reshape(TPC, 128, D).transpose(1, 0, 2).reshape(NPC, D))
        hT_blk = np.ascontiguousarray(hc.T)  # [128, NPC] bf16
        bp2 = np.stack([np.ones(NPC, np.float32), p[rows]]).astype(BF16)
        per_core.append(dict(
            h_blk=h_blk,
            hT_blk=hT_blk,
            h_gather=h_bf16,
            idx_all=idx_all,
            d_cols=np.ascontiguousarray(d_cols[c]),
            s2_cols=np.ascontiguousarray(
                s2[rows].reshape(TPC, 128).T),       # [128, TPC]
            bp2=np.ascontiguousarray(bp2),           # [2, NPC] bf16
            rhs_A=rhs_A, rhs_B=rhs_B, bias2=bias2,
            iota=np.ascontiguousarray(
                np.broadcast_to(np.arange(128, dtype=np.float32), (128, 128))
            ).astype(BF16),
        ))
    meta = dict(caps=caps, S_t=S_t, cap_off=cap_off, trivial_ln=trivial_ln,
                ln_gamma=np.asarray(ln_gamma, np.float32),
                ln_beta=np.asarray(ln_beta, np.float32))
    return per_core, meta


# ---------------------------------------------------------------------------
# Bass device kernel
# ---------------------------------------------------------------------------

_BASS_CACHE = {}


def _build_bass(caps, S_t, cap_off, trivial_ln, idxcols):
    import concourse.bacc as bacc
    import concourse.bass as bass
    import concourse.tile as tile
    from concourse import mybir

    f32 = mybir.dt.float32
    bf16 = mybir.dt.bfloat16
    fp8 = mybir.dt.float8e4
    i16 = mybir.dt.int16
    Alu = mybir.AluOpType
    Act = mybir.ActivationFunctionType

    nc = bacc.Bacc("TRN2", target_bir_lowering=False, debug=False,
                   num_devices=N_CORES)

    h_blk_d = nc.dram_tensor("h_blk", [NPC, D], bf16, kind="ExternalInput")
    hT_blk_d = nc.dram_tensor("hT_blk", [D, NPC], bf16, kind="ExternalInput")
    h_gather = nc.dram_tensor("h_gather", [N_PAD, D], bf16, kind="ExternalInput")
    idx_all = nc.dram_tensor("idx_all", [128, idxcols], i16, kind="ExternalInput")
    d_cols_d = nc.dram_tensor("d_cols", [128, TPC * S_t], f32, kind="ExternalInput")
    s2_cols_d = nc.dram_tensor("s2_cols", [128, TPC], f32, kind="ExternalInput")
    bp2_d = nc.dram_tensor("bp2", [2, NPC], bf16, kind="ExternalInput")
    rhs_A_d = nc.dram_tensor("rhs_A", [D, 2 * D], bf16, kind="ExternalInput")
    rhs_B_d = nc.dram_tensor("rhs_B", [D, 2 * D], bf16, kind="ExternalInput")
    bias2_d = nc.dram_tensor("bias2", [2, 2 * D], bf16, kind="ExternalInput")
    iota_d = nc.dram_tensor("iota", [128, 128], bf16, kind="ExternalInput")
    out_d = nc.dram_tensor("out", [NPC, D], bf16, kind="ExternalOutput")

    b_off = [TG * sum(caps[:b]) for b in range(N_BANKS)]  # G slot region start
    seg_w = [TG * caps[b] * 128 // 16 for b in range(N_BANKS)]  # idx cols/(g,b)

    with tile.TileContext(nc) as tc:
        with (
            tc.tile_pool(name="consts", bufs=1) as cpool,
            tc.tile_pool(name="gbuf", bufs=2) as gpool,
            tc.tile_pool(name="ptbuf", bufs=2) as ptpool,
            tc.tile_pool(name="vbuf", bufs=2) as vpool,
            tc.tile_pool(name="work", bufs=3) as wpool,
            tc.tile_pool(name="small", bufs=2) as spool,
            tc.tile_pool(name="psS", bufs=2, space="PSUM") as psS,
            tc.tile_pool(name="psA", bufs=2, space="PSUM") as psA,
            tc.tile_pool(name="psB", bufs=2, space="PSUM") as psB,
        ):
            # --- constants, loaded once ---
            idx_sb = cpool.tile([128, idxcols], i16)
            nc.sync.dma_start(out=idx_sb[:], in_=idx_all[:])
            d_cols_sb = cpool.tile([128, TPC * S_t], f32)
            nc.sync.dma_start(out=d_cols_sb[:], in_=d_cols_d[:])
            s2_sb = cpool.tile([128, TPC], f32)
            nc.sync.dma_start(out=s2_sb[:], in_=s2_cols_d[:])
            bp2_sb = cpool.tile([2, NPC], bf16)
            nc.sync.dma_start(out=bp2_sb[:], in_=bp2_d[:])
            rhsA_sb = cpool.tile([D, 2 * D], bf16)
            nc.sync.dma_start(out=rhsA_sb[:], in_=rhs_A_d[:])
            rhsB_sb = cpool.tile([D, 2 * D], bf16)
            nc.sync.dma_start(out=rhsB_sb[:], in_=rhs_B_d[:])
            bias2_sb = cpool.tile([2, 2 * D], bf16)
            nc.sync.dma_start(out=bias2_sb[:], in_=bias2_d[:])
            iota_sb = cpool.tile([128, 128], bf16)
            nc.sync.dma_start(out=iota_sb[:], in_=iota_d[:])
            eps_sb = cpool.tile([128, 1], f32)
            nc.vector.memset(eps_sb[:], LN_EPS)
            # whole-core h / hT / out blocks, SBUF-resident
            h_all = cpool.tile([128, NPC], bf16)
            nc.sync.dma_start(
                out=h_all[:],
                in_=bass.AP(tensor=h_blk_d, offset=0,
                            ap=[[TPC * D, 128], [1, TPC * D]]))
            hT_all = cpool.tile([128, NPC], bf16)
            nc.sync.dma_start(out=hT_all[:], in_=hT_blk_d[:])
            out_all = cpool.tile([128, NPC], bf16)
            if not trivial_ln:
                gamma_sb = cpool.tile([128, D], f32)
                nc.gpsimd.dma_start(
                    out=gamma_sb[:],
                    in_=bass.AP(tensor=nc.dram_tensor(
                        "ln_gamma", [1, D], f32, kind="ExternalInput"),
                        offset=0, ap=[[0, 128], [1, D]]))
                beta_sb = cpool.tile([128, D], f32)
                nc.gpsimd.dma_start(
                    out=beta_sb[:],
                    in_=bass.AP(tensor=nc.dram_tensor(
                        "ln_beta", [1, D], f32, kind="ExternalInput"),
                        offset=0, ap=[[0, 128], [1, D]]))

            idx_off = [0]
            for g in range(N_GROUPS):
                for b in range(N_BANKS):
                    idx_off.append(idx_off[-1] + seg_w[b])

            for g in range(N_GROUPS):
                G = gpool.tile([128, TG * S_t, 128], bf16, tag="G")
                for b in range(N_BANKS):
                    o = idx_off[g * N_BANKS + b]
                    nidx = TG * caps[b] * 128
                    nc.gpsimd.dma_gather(
                        out_ap=G[:, b_off[b] : b_off[b] + TG * caps[b], :],
                        in_ap=h_gather[b * BANK :, :],
                        idxs_ap=idx_sb[:, o : o + seg_w[b]],
                        num_idxs=nidx,
                        num_idxs_reg=nidx,
                        elem_size=D,
                        single_packet=False,
                    )
                # group-level buffers
                v_g = vpool.tile([128, TG * D], bf16, tag="v")
                sum_g = spool.tile([128, TG], f32, tag="sumv")
                sq_g = spool.tile([128, TG], f32, tag="sv2")
                # ---------------- phase A: gate + v ----------------
                for ti in range(TG):
                    t = g * TG + ti
                    tc0, tc1 = t * 128, (t + 1) * 128
                    # one-hot P^T for the slots
                    PT = ptpool.tile([128, S_t, 128], fp8, tag="PT")
                    for k in range(S_t):
                        col = t * S_t + k
                        nc.vector.tensor_scalar(
                            out=PT[:, k, :], in0=iota_sb[:],
                            scalar1=d_cols_sb[:, col : col + 1],
                            scalar2=None, op0=Alu.is_equal)
                    # segment sum: segT [dim, dst]
                    ps_S = psS.tile([128, 128], f32, tag="psS")
                    nmm = 0
                    for b in range(N_BANKS):
                        for j in range(caps[b]):
                            slot = b_off[b] + ti * caps[b] + j
                            k = cap_off[b] + j
                            nc.tensor.matmul(
                                ps_S[:], lhsT=G[:, slot, :], rhs=PT[:, k, :],
                                start=(nmm == 0), stop=(nmm == S_t - 1))
                            nmm += 1
                    segT = wpool.tile([128, 128], bf16, tag="segT")
                    nc.scalar.copy(out=segT[:], in_=ps_S[:])

                    # dense matmuls
                    ps_A = psA.tile([128, 2 * D], f32, tag="psA")
                    nc.tensor.matmul(ps_A[:], lhsT=hT_all[:, tc0:tc1],
                                     rhs=rhsA_sb[:], start=True, stop=False)
                    nc.tensor.matmul(ps_A[:], lhsT=bp2_sb[:, tc0:tc1],
                                     rhs=bias2_sb[:], start=False, stop=True)
                    ps_B = psB.tile([128, 2 * D], f32, tag="psB")
                    nc.tensor.matmul(ps_B[:], lhsT=segT[:], rhs=rhsB_sb[:],
                                     start=True, stop=True)

                    # Y = s2 * ps_B + ps_A  (scalar applies the per-node s2
                    # scale while moving ps_B out of PSUM; DVE adds ps_A)
                    Bs2 = wpool.tile([128, 2 * D], bf16, tag="Bs2")
                    nc.scalar.mul(out=Bs2[:], in_=ps_B[:],
                                  mul=s2_sb[:, t : t + 1])
                    Y = wpool.tile([128, 2 * D], bf16, tag="Y")
                    nc.vector.tensor_tensor(out=Y[:], in0=ps_A[:], in1=Bs2[:],
                                            op=Alu.add)
                    h_t = h_all[:, tc0:tc1]
                    g_sb = wpool.tile([128, D], bf16, tag="g_sb")
                    nc.scalar.activation(out=g_sb[:], in_=Y[:, D:],
                                         func=Act.Sigmoid)
                    t1 = wpool.tile([128, D], bf16, tag="t1")
                    nc.vector.tensor_tensor(out=t1[:], in0=Y[:, :D], in1=h_t,
                                            op=Alu.subtract)
                    t2 = wpool.tile([128, D], bf16, tag="t2")
                    nc.vector.tensor_tensor(out=t2[:], in0=g_sb[:], in1=t1[:],
                                            op=Alu.mult)
                    v_sl = v_g[:, ti * D : (ti + 1) * D]
                    nc.vector.scalar_tensor_tensor(
                        out=v_sl, in0=t2[:], scalar=0.0, in1=h_t,
                        op0=Alu.add, op1=Alu.add,
                        accum_out=sum_g[:, ti : ti + 1])
                    vsq = wpool.tile([128, D], bf16, tag="vsq")
                    nc.scalar.activation(out=vsq[:], in_=v_sl,
                                         func=Act.Square,
                                         accum_out=sq_g[:, ti : ti + 1])
                # ---------------- phase B: layernorm (batched stats) -------
                mu_g = spool.tile([128, TG], f32, tag="mu")
                nc.vector.tensor_scalar(out=mu_g[:], in0=sum_g[:],
                                        scalar1=1.0 / D, scalar2=None,
                                        op0=Alu.mult)
                msq_g = spool.tile([128, TG], f32, tag="msq")
                nc.vector.tensor_scalar(out=msq_g[:], in0=sq_g[:],
                                        scalar1=1.0 / D, scalar2=None,
                                        op0=Alu.mult)
                musq_g = spool.tile([128, TG], f32, tag="musq")
                nc.vector.tensor_tensor(out=musq_g[:], in0=mu_g[:],
                                        in1=mu_g[:], op=Alu.mult)
                var_g = spool.tile([128, TG], f32, tag="var")
                nc.vector.tensor_tensor(out=var_g[:], in0=msq_g[:],
                                        in1=musq_g[:], op=Alu.subtract)
                std_g = spool.tile([128, TG], f32, tag="std")
                nc.scalar.activation(out=std_g[:], in_=var_g[:], func=Act.Sqrt,
                                     bias=eps_sb[:])
                rstd_g = spool.tile([128, TG], f32, tag="rstd")
                nc.vector.reciprocal(out=rstd_g[:], in_=std_g[:])
                bl_g = spool.tile([128, TG], f32, tag="bl")
                nc.vector.scalar_tensor_tensor(
                    out=bl_g[:], in0=mu_g[:], scalar=-1.0, in1=rstd_g[:],
                    op0=Alu.mult, op1=Alu.mult)
                for ti in range(TG):
                    t = g * TG + ti
                    tc0, tc1 = t * 128, (t + 1) * 128
                    v_sl = v_g[:, ti * D : (ti + 1) * D]
                    if trivial_ln:
                        nc.scalar.activation(out=out_all[:, tc0:tc1],
                                             in_=v_sl, func=Act.Relu,
                                             bias=bl_g[:, ti : ti + 1],
                                             scale=rstd_g[:, ti : ti + 1])
                    else:
                        z = wpool.tile([128, D], f32, tag="z")
                        nc.scalar.activation(out=z[:], in_=v_sl,
                                             func=Act.Identity,
                                             bias=bl_g[:, ti : ti + 1],
                                             scale=rstd_g[:, ti : ti + 1])
                        nc.vector.tensor_tensor(out=z[:], in0=z[:],
                                                in1=gamma_sb[:], op=Alu.mult)
                        nc.vector.tensor_tensor(out=z[:], in0=z[:],
                                                in1=beta_sb[:], op=Alu.add)
                        nc.scalar.activation(out=out_all[:, tc0:tc1],
                                             in_=z[:], func=Act.Relu)
            # one big store at the end (host un-permutes)
            nc.sync.dma_start(
                out=bass.AP(tensor=out_d, offset=0,
                            ap=[[TPC * D, 128], [1, TPC * D]]),
                in_=out_all[:])
    nc.compile()
    return nc


def kernel(**inputs):
    from concourse.bass_utils import run_bass_kernel_spmd

    per_core, meta = _prep(**{k: np.asarray(v) for k, v in inputs.items()})
    idxcols = per_core[0]["idx_all"].shape[1]
    key = (tuple(meta["caps"]), meta["trivial_ln"], idxcols)
    if key not in _BASS_CACHE:
        _BASS_CACHE[key] = _build_bass(
            meta["caps"], meta["S_t"], meta["cap_off"], meta["trivial_ln"],
            idxcols)
    nc = _BASS_CACHE[key]

    in_maps = []
    for pc in per_core:
        m = dict(pc)
        if not meta["trivial_ln"]:
            m["ln_gamma"] = meta["ln_gamma"][None]
            m["ln_beta"] = meta["ln_beta"][None]
        in_maps.append(m)
    res = run_bass_kernel_spmd(nc, in_maps, core_ids=list(range(N_CORES)))
    outs = []
    for c in range(N_CORES):
        o = np.asarray(res.results[c]["out"])  # [NPC, D] bf16, block-permuted
        o = o.reshape(128, TPC, D).transpose(1, 0, 2).reshape(NPC, D)
        outs.append(o.astype(np.float32))
    return np.concatenate(outs, 0)[:N]


# revision 9
# speedup vs baseline: 1.0059x; 1.0059x over previous
"""GNN message-passing layer (DAGLayer) on 8 Trainium2 NeuronCores.

Strategy (v3):
  - Shard destination nodes 8 ways (12544 nodes = 98 tiles of 128 per core,
    N padded 100000 -> 100352). Replicate h (as a bf16 gather table) so
    cross-partition edges need no communication.
  - Segment-sum over edges via TensorE matmuls: for each 128-dst tile,
    gather h[src] rows (dma_gather, bf16) for its edges in chunks of 128,
    build one-hot P^T on DVE (single-op is_equal vs iota, bf16),
    accumulate segT = G^T @ P^T in PSUM ([dim, dst] layout -> lhsT reuse).
  - Edge indices are int16 for dma_gather, so h is split into 4 row-banks
    of 25088; each (tile, bank) gets a fixed number of 128-edge chunk
    slots (cap_b, normally 2) so the compiled program is identical on all
    cores (SPMD) -- only the data (indices, one-hot columns) differs.
  - Dense path per tile with host-folded weights, all-bf16 matmuls:
      ps_A = h @ [Ws^T | (Wg1+Wg2@Ws)^T]
           + [1|p] @ [[b_s|b_g+Wg2@b_s]; [b_n|Wg2@b_n]]   (K=2 matmul)
      ps_B = seg @ [Wn^T | (Wg2@Wn)^T]
      Y    = s2 * ps_B + ps_A        (DVE stt; s2 = has_pred/max(cnt,1))
      m, gp = Y[:, :128], Y[:, 128:]
      v = h + sigmoid(gp) * (m - h)
      out = relu((v - mu) * rstd)    [bf16, cast to f32 on host]
  - Per-group two-phase scalar schedule to avoid ACT_TABLE_LOAD thrash:
    phase A (gate) uses {Sigmoid, Square, Copy} = sigmoid table; v is
    buffered per group; phase B (layernorm) uses {Sqrt, Relu} = sqrt
    table; LN statistics are computed batched [128, TG] per group.
  - h / hT / out live SBUF-resident for the whole core, loaded/stored via
    three large DMAs (host permutes rows so the block DMA lands node
    t*128+p on partition p).
"""

import numpy as np

try:
    import ml_dtypes

    BF16 = np.dtype(ml_dtypes.bfloat16)
except ImportError:  # pragma: no cover
    BF16 = None

N = 100000
E = 600000
D = 128
N_CORES = 8
NPC = 12544            # nodes per core
TPC = NPC // 128       # 98 tiles per core
N_PAD = NPC * N_CORES  # 100352
N_BANKS = 4
BANK = N_PAD // N_BANKS  # 25088
TG = 14                # tiles per gather group
N_GROUPS = TPC // TG   # 7
LN_EPS = 1e-5


def _build_schedule(edge_src, edge_dst):
    """Chunk/slot schedule. Returns per-core idx16 + d_cols arrays and the
    (core-independent) slot capacities."""
    edge_src = np.asarray(edge_src, np.int64)
    edge_dst = np.asarray(edge_dst, np.int64)

    counts = np.bincount(edge_dst, minlength=N_PAD)

    tile_g = edge_dst // 128          # global dst tile 0..783
    bank = edge_src // BANK           # 0..3
    key = tile_g * N_BANKS + bank
    order = np.argsort(key, kind="stable")
    src_s = edge_src[order]
    dst_s = edge_dst[order]

    n_tiles_g = N_PAD // 128          # 784
    cnt = np.bincount(key[order], minlength=n_tiles_g * N_BANKS).reshape(
        n_tiles_g, N_BANKS
    )
    starts = np.zeros((n_tiles_g, N_BANKS), np.int64)
    starts.reshape(-1)[1:] = np.cumsum(cnt.reshape(-1))[:-1]

    caps = [max(1, int(np.ceil(cnt[:, b].max() / 128.0))) for b in range(N_BANKS)]
    S_t = sum(caps)                   # slots per tile
    cap_off = np.cumsum([0] + caps)   # slot offset of bank b within a tile

    # per-core arrays
    idx16 = np.zeros((N_CORES, N_GROUPS, N_BANKS), object)
    d_cols = np.full((N_CORES, 128, TPC * S_t), -1.0, np.float32)

    for c in range(N_CORES):
        for g in range(N_GROUPS):
            for b in range(N_BANKS):
                lst = np.zeros(TG * caps[b] * 128, np.int16)
                for ti in range(TG):
                    t_local = g * TG + ti
                    t_glob = c * TPC + t_local
                    n_e = int(cnt[t_glob, b])
                    s0 = int(starts[t_glob, b])
                    srcs = src_s[s0 : s0 + n_e] - b * BANK
                    dsts = dst_s[s0 : s0 + n_e] - t_glob * 128
                    base = ti * caps[b] * 128
                    lst[base : base + n_e] = srcs.astype(np.int16)
                    for j in range(caps[b]):
                        col = t_local * S_t + cap_off[b] + j
                        seg = dsts[j * 128 : (j + 1) * 128]
                        d_cols[c, : len(seg), col] = seg
                idx16[c, g, b] = lst
    return idx16, d_cols, caps, S_t, cap_off, counts


def _wrap_idx16(flat):
    """dma_gather index layout: idx i -> [i % 16, i // 16], replicated to
    all 128 partitions."""
    n = len(flat)
    w = np.zeros((128, n // 16), np.int16)
    w16 = flat.reshape(n // 16, 16).T  # [16, n/16]
    w[:16] = w16
    w[16:] = np.tile(w16, (7, 1))
    return w


def _fold_weights(W_self, b_self, W_neigh, b_neigh, W_gate, b_gate):
    Wg1 = W_gate[:, :D]
    Wg2 = W_gate[:, D:]
    rhs_A = np.concatenate([W_self.T, (Wg1 + Wg2 @ W_self).T], 1)  # [128,256]
    rhs_B = np.concatenate([W_neigh.T, (Wg2 @ W_neigh).T], 1)      # [128,256]
    bias_A = np.concatenate([b_self, b_gate + Wg2 @ b_self])        # [256]
    bias_Bp = np.concatenate([b_neigh, Wg2 @ b_neigh])              # [256]
    bias2 = np.stack([bias_A, bias_Bp])                             # [2,256]
    return (
        np.ascontiguousarray(rhs_A).astype(BF16),
        np.ascontiguousarray(rhs_B).astype(BF16),
        np.ascontiguousarray(bias2).astype(BF16),
    )


def _prep(h, edge_src, edge_dst, W_self, b_self, W_neigh, b_neigh, W_gate,
          b_gate, ln_gamma, ln_beta):
    h = np.asarray(h, np.float32)
    h_pad = np.zeros((N_PAD, D), np.float32)
    h_pad[:N] = h
    h_bf16 = np.ascontiguousarray(h_pad).astype(BF16)

    idx16, d_cols, caps, S_t, cap_off, counts = _build_schedule(
        edge_src, edge_dst)

    p = (counts > 0).astype(np.float32)
    s2 = p / np.maximum(counts, 1).astype(np.float32)

    rhs_A, rhs_B, bias2 = _fold_weights(
        np.asarray(W_self, np.float32), np.asarray(b_self, np.float32),
        np.asarray(W_neigh, np.float32), np.asarray(b_neigh, np.float32),
        np.asarray(W_gate, np.float32), np.asarray(b_gate, np.float32))

    trivial_ln = (np.allclose(ln_gamma, 1.0) and np.allclose(ln_beta, 0.0))

    per_core = []
    for c in range(N_CORES):
        rows = slice(c * NPC, (c + 1) * NPC)
        idx_segs = []
        for g in range(N_GROUPS):
            for b in range(N_BANKS):
                idx_segs.append(_wrap_idx16(idx16[c, g, b]))
        idx_all = np.concatenate(idx_segs, axis=1)  # [128, sum(n/16)]
        hc = h_bf16[rows]  # [NPC, 128] bf16
        # block-permuted rows: DRAM row p*TPC + t holds node t*128+p, so one
        # contiguous DMA lands node t*128+p at SBUF [p, t*128:(t+1)*128]
        h_blk = np.ascontiguousarray(
            hc.reshape(TPC, 128, D).transpose(1, 0, 2).reshape(NPC, D))
        hT_blk = np.ascontiguousarray(hc.T)  # [128, NPC] bf16
        bp2 = np.stack([np.ones(NPC, np.float32), p[rows]]).astype(BF16)
        per_core.append(dict(
            h_blk=h_blk,
            hT_blk=hT_blk,
            h_gather=h_bf16,
            idx_all=idx_all,
            d_cols=np.ascontiguousarray(d_cols[c]),
            s2_cols=np.ascontiguousarray(
                s2[rows].reshape(TPC, 128).T),       # [128, TPC]
            bp2=np.ascontiguousarray(bp2),           # [2, NPC] bf16
            rhs_A=rhs_A, rhs_B=rhs_B, bias2=bias2,
            iota=np.ascontiguousarray(
                np.broadcast_to(np.arange(128, dtype=np.float32), (128, 128))
            ).astype(BF16),
        ))
    meta = dict(caps=caps, S_t=S_t, cap_off=cap_off, trivial_ln=trivial_ln,
                ln_gamma=np.asarray(ln_gamma, np.float32),
                ln_beta=np.asarray(ln_beta, np.float32))
    return per_core, meta


# ---------------------------------------------------------------------------
# Bass device kernel
# ---------------------------------------------------------------------------

_BASS_CACHE = {}


def _build_bass(caps, S_t, cap_off, trivial_ln, idxcols):
    import concourse.bacc as bacc
    import concourse.bass as bass
    import concourse.tile as tile
    from concourse import mybir

    f32 = mybir.dt.float32
    bf16 = mybir.dt.bfloat16
    fp8 = mybir.dt.float8e4
    i16 = mybir.dt.int16
    Alu = mybir.AluOpType
    Act = mybir.ActivationFunctionType

    nc = bacc.Bacc("TRN2", target_bir_lowering=False, debug=False,
                   num_devices=N_CORES)

    h_blk_d = nc.dram_tensor("h_blk", [NPC, D], bf16, kind="ExternalInput")
    hT_blk_d = nc.dram_tensor("hT_blk", [D, NPC], bf16, kind="ExternalInput")
    h_gather = nc.dram_tensor("h_gather", [N_PAD, D], bf16, kind="ExternalInput")
    idx_all = nc.dram_tensor("idx_all", [128, idxcols], i16, kind="ExternalInput")
    d_cols_d = nc.dram_tensor("d_cols", [128, TPC * S_t], f32, kind="ExternalInput")
    s2_cols_d = nc.dram_tensor("s2_cols", [128, TPC], f32, kind="ExternalInput")
    bp2_d = nc.dram_tensor("bp2", [2, NPC], bf16, kind="ExternalInput")
    rhs_A_d = nc.dram_tensor("rhs_A", [D, 2 * D], bf16, kind="ExternalInput")
    rhs_B_d = nc.dram_tensor("rhs_B", [D, 2 * D], bf16, kind="ExternalInput")
    bias2_d = nc.dram_tensor("bias2", [2, 2 * D], bf16, kind="ExternalInput")
    iota_d = nc.dram_tensor("iota", [128, 128], bf16, kind="ExternalInput")
    out_d = nc.dram_tensor("out", [NPC, D], bf16, kind="ExternalOutput")

    b_off = [TG * sum(caps[:b]) for b in range(N_BANKS)]  # G slot region start
    seg_w = [TG * caps[b] * 128 // 16 for b in range(N_BANKS)]  # idx cols/(g,b)

    with tile.TileContext(nc) as tc:
        with (
            tc.tile_pool(name="consts", bufs=1) as cpool,
            tc.tile_pool(name="gbuf", bufs=2) as gpool,
            tc.tile_pool(name="ptbuf", bufs=2) as ptpool,
            tc.tile_pool(name="vbuf", bufs=2) as vpool,
            tc.tile_pool(name="work", bufs=3) as wpool,
            tc.tile_pool(name="small", bufs=2) as spool,
            tc.tile_pool(name="psS", bufs=2, space="PSUM") as psS,
            tc.tile_pool(name="psA", bufs=2, space="PSUM") as psA,
            tc.tile_pool(name="psB", bufs=2, space="PSUM") as psB,
        ):
            # --- constants, loaded once ---
            idx_sb = cpool.tile([128, idxcols], i16)
            nc.sync.dma_start(out=idx_sb[:], in_=idx_all[:])
            d_cols_sb = cpool.tile([128, TPC * S_t], f32)
            nc.sync.dma_start(out=d_cols_sb[:], in_=d_cols_d[:])
            s2_sb = cpool.tile([128, TPC], f32)
            nc.sync.dma_start(out=s2_sb[:], in_=s2_cols_d[:])
            bp2_sb = cpool.tile([2, NPC], bf16)
            nc.sync.dma_start(out=bp2_sb[:], in_=bp2_d[:])
            rhsA_sb = cpool.tile([D, 2 * D], bf16)
            nc.sync.dma_start(out=rhsA_sb[:], in_=rhs_A_d[:])
            rhsB_sb = cpool.tile([D, 2 * D], bf16)
            nc.sync.dma_start(out=rhsB_sb[:], in_=rhs_B_d[:])
            bias2_sb = cpool.tile([2, 2 * D], bf16)
            nc.sync.dma_start(out=bias2_sb[:], in_=bias2_d[:])
            iota_sb = cpool.tile([128, 128], bf16)
            nc.sync.dma_start(out=iota_sb[:], in_=iota_d[:])
            eps_sb = cpool.tile([128, 1], f32)
            nc.vector.memset(eps_sb[:], LN_EPS)
            # whole-core h / hT / out blocks, SBUF-resident
            h_all = cpool.tile([128, NPC], bf16)
            nc.sync.dma_start(
                out=h_all[:],
                in_=bass.AP(tensor=h_blk_d, offset=0,
                            ap=[[TPC * D, 128], [1, TPC * D]]))
            hT_all = cpool.tile([128, NPC], bf16)
            nc.sync.dma_start(out=hT_all[:], in_=hT_blk_d[:])
            out_all = cpool.tile([128, NPC], bf16)
            if not trivial_ln:
                gamma_sb = cpool.tile([128, D], f32)
                nc.gpsimd.dma_start(
                    out=gamma_sb[:],
                    in_=bass.AP(tensor=nc.dram_tensor(
                        "ln_gamma", [1, D], f32, kind="ExternalInput"),
                        offset=0, ap=[[0, 128], [1, D]]))
                beta_sb = cpool.tile([128, D], f32)
                nc.gpsimd.dma_start(
                    out=beta_sb[:],
                    in_=bass.AP(tensor=nc.dram_tensor(
                        "ln_beta", [1, D], f32, kind="ExternalInput"),
                        offset=0, ap=[[0, 128], [1, D]]))

            idx_off = [0]
            for g in range(N_GROUPS):
                for b in range(N_BANKS):
                    idx_off.append(idx_off[-1] + seg_w[b])

            for g in range(N_GROUPS):
                G = gpool.tile([128, TG * S_t, 128], bf16, tag="G")
                for b in range(N_BANKS):
                    o = idx_off[g * N_BANKS + b]
                    nidx = TG * caps[b] * 128
                    nc.gpsimd.dma_gather(
                        out_ap=G[:, b_off[b] : b_off[b] + TG * caps[b], :],
                        in_ap=h_gather[b * BANK :, :],
                        idxs_ap=idx_sb[:, o : o + seg_w[b]],
                        num_idxs=nidx,
                        num_idxs_reg=nidx,
                        elem_size=D,
                        single_packet=False,
                    )
                # group-level buffers
                v_g = vpool.tile([128, TG * D], bf16, tag="v")
                sum_g = spool.tile([128, TG], f32, tag="sumv")
                sq_g = spool.tile([128, TG], f32, tag="sv2")
                # ---------------- phase A: gate + v ----------------
                for ti in range(TG):
                    t = g * TG + ti
                    tc0, tc1 = t * 128, (t + 1) * 128
                    # one-hot P^T for the slots
                    PT = ptpool.tile([128, S_t, 128], fp8, tag="PT")
                    for k in range(S_t):
                        col = t * S_t + k
                        nc.vector.tensor_scalar(
                            out=PT[:, k, :], in0=iota_sb[:],
                            scalar1=d_cols_sb[:, col : col + 1],
                            scalar2=None, op0=Alu.is_equal)
                    # segment sum: segT [dim, dst]
                    ps_S = psS.tile([128, 128], f32, tag="psS")
                    nmm = 0
                    for b in range(N_BANKS):
                        for j in range(caps[b]):
                            slot = b_off[b] + ti * caps[b] + j
                            k = cap_off[b] + j
                            nc.tensor.matmul(
                                ps_S[:], lhsT=G[:, slot, :], rhs=PT[:, k, :],
                                start=(nmm == 0), stop=(nmm == S_t - 1))
                            nmm += 1
                    segT = wpool.tile([128, 128], bf16, tag="segT")
                    nc.scalar.copy(out=segT[:], in_=ps_S[:])

                    # dense matmuls
                    ps_A = psA.tile([128, 2 * D], f32, tag="psA")
                    nc.tensor.matmul(ps_A[:], lhsT=hT_all[:, tc0:tc1],
                                     rhs=rhsA_sb[:], start=True, stop=False)
                    nc.tensor.matmul(ps_A[:], lhsT=bp2_sb[:, tc0:tc1],
                                     rhs=bias2_sb[:], start=False, stop=True)
                    ps_B = psB.tile([128, 2 * D], f32, tag="psB")
                    nc.tensor.matmul(ps_B[:], lhsT=segT[:], rhs=rhsB_sb[:],
                                     start=True, stop=True)

                    # Y = s2 * ps_B + ps_A  (scalar applies the per-node s2
                    # scale while moving ps_B out of PSUM; DVE adds ps_A)
                    Bs2 = wpool.tile([128, 2 * D], bf16, tag="Bs2")
                    nc.scalar.mul(out=Bs2[:], in_=ps_B[:],
                                  mul=s2_sb[:, t : t + 1])
                    Y = wpool.tile([128, 2 * D], bf16, tag="Y")
                    nc.vector.tensor_tensor(out=Y[:], in0=ps_A[:], in1=Bs2[:],
                                            op=Alu.add)
                    h_t = h_all[:, tc0:tc1]
                    g_sb = wpool.tile([128, D], bf16, tag="g_sb")
                    nc.scalar.activation(out=g_sb[:], in_=Y[:, D:],
                                         func=Act.Sigmoid)
                    t1 = wpool.tile([128, D], bf16, tag="t1")
                    nc.vector.tensor_tensor(out=t1[:], in0=Y[:, :D], in1=h_t,
                                            op=Alu.subtract)
                    t2 = wpool.tile([128, D], bf16, tag="t2")
                    nc.vector.tensor_tensor(out=t2[:], in0=g_sb[:], in1=t1[:],
                                            op=Alu.mult)
                    v_sl = v_g[:, ti * D : (ti + 1) * D]
                    nc.vector.scalar_tensor_tensor(
                        out=v_sl, in0=t2[:], scalar=0.0, in1=h_t,
                        op0=Alu.add, op1=Alu.add,
                        accum_out=sum_g[:, ti : ti + 1])
                    vsq = wpool.tile([128, D], bf16, tag="vsq")
                    nc.scalar.activation(out=vsq[:], in_=v_sl,
                                         func=Act.Square,
                                         accum_out=sq_g[:, ti : ti + 1])
                # ---------------- phase B: layernorm (batched stats) -------
                mu_g = spool.tile([128, TG], f32, tag="mu")
                nc.vector.tensor_scalar(out=mu_g[:], in0=sum_g[:],
                                        scalar1=1.0 / D, scalar2=None,
                                        op0=Alu.mult)
                msq_g = spool.tile([128, TG], f32, tag="msq")
                nc.vector.tensor_scalar(out=msq_g[:], in0=sq_g[:],
                                        scalar1=1.0 / D, scalar2=None,
                                        op0=Alu.mult)
                musq_g = spool.tile([128, TG], f32, tag="musq")
                nc.vector.tensor_tensor(out=musq_g[:], in0=mu_g[:],
                                        in1=mu_g[:], op=Alu.mult)
                var_g = spool.tile([128, TG], f32, tag="var")
                nc.vector.tensor_tensor(out=var_g[:], in0=msq_g[:],
                                        in1=musq_g[:], op=Alu.subtract)
                std_g = spool.tile([128, TG], f32, tag="std")
                nc.scalar.activation(out=std_g[:], in_=var_g[:], func=Act.Sqrt,
                                     bias=eps_sb[:])
                rstd_g = spool.tile([128, TG], f32, tag="rstd")
                nc.vector.reciprocal(out=rstd_g[:], in_=std_g[:])
                bl_g = spool.tile([128, TG], f32, tag="bl")
                nc.vector.scalar_tensor_tensor(
                    out=bl_g[:], in0=mu_g[:], scalar=-1.0, in1=rstd_g[:],
                    op0=Alu.mult, op1=Alu.mult)
                for ti in range(TG):
                    t = g * TG + ti
                    tc0, tc1 = t * 128, (t + 1) * 128
                    v_sl = v_g[:, ti * D : (ti + 1) * D]
                    if trivial_ln:
                        nc.scalar.activation(out=out_all[:, tc0:tc1],
                                             in_=v_sl, func=Act.Relu,
                                             bias=bl_g[:, ti : ti + 1],
                                             scale=rstd_g[:, ti : ti + 1])
                    else:
                        z = wpool.tile([128, D], f32, tag="z")
                        nc.scalar.activation(out=z[:], in_=v_sl,
                                             func=Act.Identity,
                                             bias=bl_g[:, ti : ti + 1],
                                             scale=rstd_g[:, ti : ti + 1])
                        nc.vector.tensor_tensor(out=z[:], in0=z[:],
                                                in1=gamma_sb[:], op=Alu.mult)
                        nc.vector.tensor_tensor(out=z[:], in0=z[:],
                                                in1=beta_sb[:], op=Alu.add)
                        nc.scalar.activation(out=out_all[:, tc0:tc1],
                                             in_=z[:], func=Act.Relu)
                # store this group's output slice (host un-permutes)
                g0 = g * TG * D
                nc.sync.dma_start(
                    out=bass.AP(tensor=out_d, offset=g0,
                                ap=[[TPC * D, 128], [1, TG * D]]),
                    in_=out_all[:, g0 : g0 + TG * D])
    nc.compile()
    return nc


def kernel(**inputs):
    from concourse.bass_utils import run_bass_kernel_spmd

    per_core, meta = _prep(**{k: np.asarray(v) for k, v in inputs.items()})
    idxcols = per_core[0]["idx_all"].shape[1]
    key = (tuple(meta["caps"]), meta["trivial_ln"], idxcols)
    if key not in _BASS_CACHE:
        _BASS_CACHE[key] = _build_bass(
            meta["caps"], meta["S_t"], meta["cap_off"], meta["trivial_ln"],
            idxcols)
    nc = _BASS_CACHE[key]

    in_maps = []
    for pc in per_core:
        m = dict(pc)
        if not meta["trivial_ln"]:
            m["ln_gamma"] = meta["ln_gamma"][None]
            m["ln_beta"] = meta["ln_beta"][None]
        in_maps.append(m)
    res = run_bass_kernel_spmd(nc, in_maps, core_ids=list(range(N_CORES)))
    outs = []
    for c in range(N_CORES):
        o = np.asarray(res.results[c]["out"])  # [NPC, D] bf16, block-permuted
        o = o.reshape(128, TPC, D).transpose(1, 0, 2).reshape(NPC, D)
        outs.append(o.astype(np.float32))
    return np.concatenate(outs, 0)[:N]


# revision 11
# speedup vs baseline: 1.0079x; 1.0021x over previous
"""GNN message-passing layer (DAGLayer) on 8 Trainium2 NeuronCores.

Strategy (v3):
  - Shard destination nodes 8 ways (12544 nodes = 98 tiles of 128 per core,
    N padded 100000 -> 100352). Replicate h (as a bf16 gather table) so
    cross-partition edges need no communication.
  - Segment-sum over edges via TensorE matmuls: for each 128-dst tile,
    gather h[src] rows (dma_gather, bf16) for its edges in chunks of 128,
    build one-hot P^T on DVE (single-op is_equal vs iota, bf16),
    accumulate segT = G^T @ P^T in PSUM ([dim, dst] layout -> lhsT reuse).
  - Edge indices are int16 for dma_gather, so h is split into 4 row-banks
    of 25088; each (tile, bank) gets a fixed number of 128-edge chunk
    slots (cap_b, normally 2) so the compiled program is identical on all
    cores (SPMD) -- only the data (indices, one-hot columns) differs.
  - Dense path per tile with host-folded weights, all-bf16 matmuls:
      ps_A = h @ [Ws^T | (Wg1+Wg2@Ws)^T]
           + [1|p] @ [[b_s|b_g+Wg2@b_s]; [b_n|Wg2@b_n]]   (K=2 matmul)
      ps_B = seg @ [Wn^T | (Wg2@Wn)^T]
      Y    = s2 * ps_B + ps_A        (DVE stt; s2 = has_pred/max(cnt,1))
      m, gp = Y[:, :128], Y[:, 128:]
      v = h + sigmoid(gp) * (m - h)
      out = relu((v - mu) * rstd)    [bf16, cast to f32 on host]
  - Per-group two-phase scalar schedule to avoid ACT_TABLE_LOAD thrash:
    phase A (gate) uses {Sigmoid, Square, Copy} = sigmoid table; v is
    buffered per group; phase B (layernorm) uses {Sqrt, Relu} = sqrt
    table; LN statistics are computed batched [128, TG] per group.
  - h / hT / out live SBUF-resident for the whole core, loaded/stored via
    three large DMAs (host permutes rows so the block DMA lands node
    t*128+p on partition p).
"""

import numpy as np

try:
    import ml_dtypes

    BF16 = np.dtype(ml_dtypes.bfloat16)
except ImportError:  # pragma: no cover
    BF16 = None

N = 100000
E = 600000
D = 128
N_CORES = 8
NPC = 12544            # nodes per core
TPC = NPC // 128       # 98 tiles per core
N_PAD = NPC * N_CORES  # 100352
N_BANKS = 4
BANK = N_PAD // N_BANKS  # 25088
TG = 14                # tiles per gather group
N_GROUPS = TPC // TG   # 7
LN_EPS = 1e-5


def _build_schedule(edge_src, edge_dst):
    """Chunk/slot schedule. Returns per-core idx16 + d_cols arrays and the
    (core-independent) slot capacities."""
    edge_src = np.asarray(edge_src, np.int64)
    edge_dst = np.asarray(edge_dst, np.int64)

    counts = np.bincount(edge_dst, minlength=N_PAD)

    tile_g = edge_dst // 128          # global dst tile 0..783
    bank = edge_src // BANK           # 0..3
    key = tile_g * N_BANKS + bank
    order = np.argsort(key, kind="stable")
    src_s = edge_src[order]
    dst_s = edge_dst[order]

    n_tiles_g = N_PAD // 128          # 784
    cnt = np.bincount(key[order], minlength=n_tiles_g * N_BANKS).reshape(
        n_tiles_g, N_BANKS
    )
    starts = np.zeros((n_tiles_g, N_BANKS), np.int64)
    starts.reshape(-1)[1:] = np.cumsum(cnt.reshape(-1))[:-1]

    caps = [max(1, int(np.ceil(cnt[:, b].max() / 128.0))) for b in range(N_BANKS)]
    S_t = sum(caps)                   # slots per tile
    cap_off = np.cumsum([0] + caps)   # slot offset of bank b within a tile

    # per-core arrays
    idx16 = np.zeros((N_CORES, N_GROUPS, N_BANKS), object)
    d_cols = np.full((N_CORES, 128, TPC * S_t), -1.0, np.float32)

    for c in range(N_CORES):
        for g in range(N_GROUPS):
            for b in range(N_BANKS):
                lst = np.zeros(TG * caps[b] * 128, np.int16)
                for ti in range(TG):
                    t_local = g * TG + ti
                    t_glob = c * TPC + t_local
                    n_e = int(cnt[t_glob, b])
                    s0 = int(starts[t_glob, b])
                    srcs = src_s[s0 : s0 + n_e] - b * BANK
                    dsts = dst_s[s0 : s0 + n_e] - t_glob * 128
                    base = ti * caps[b] * 128
                    lst[base : base + n_e] = srcs.astype(np.int16)
                    for j in range(caps[b]):
                        col = t_local * S_t + cap_off[b] + j
                        seg = dsts[j * 128 : (j + 1) * 128]
                        d_cols[c, : len(seg), col] = seg
                idx16[c, g, b] = lst
    return idx16, d_cols, caps, S_t, cap_off, counts


def _wrap_idx16(flat):
    """dma_gather index layout: idx i -> [i % 16, i // 16], replicated to
    all 128 partitions."""
    n = len(flat)
    w = np.zeros((128, n // 16), np.int16)
    w16 = flat.reshape(n // 16, 16).T  # [16, n/16]
    w[:16] = w16
    w[16:] = np.tile(w16, (7, 1))
    return w


def _fold_weights(W_self, b_self, W_neigh, b_neigh, W_gate, b_gate):
    Wg1 = W_gate[:, :D]
    Wg2 = W_gate[:, D:]
    rhs_A = np.concatenate([W_self.T, (Wg1 + Wg2 @ W_self).T], 1)  # [128,256]
    rhs_B = np.concatenate([W_neigh.T, (Wg2 @ W_neigh).T], 1)      # [128,256]
    bias_A = np.concatenate([b_self, b_gate + Wg2 @ b_self])        # [256]
    bias_Bp = np.concatenate([b_neigh, Wg2 @ b_neigh])              # [256]
    bias2 = np.stack([bias_A, bias_Bp])                             # [2,256]
    return (
        np.ascontiguousarray(rhs_A).astype(BF16),
        np.ascontiguousarray(rhs_B).astype(BF16),
        np.ascontiguousarray(bias2).astype(BF16),
    )


def _prep(h, edge_src, edge_dst, W_self, b_self, W_neigh, b_neigh, W_gate,
          b_gate, ln_gamma, ln_beta):
    h = np.asarray(h, np.float32)
    h_pad = np.zeros((N_PAD, D), np.float32)
    h_pad[:N] = h
    h_bf16 = np.ascontiguousarray(h_pad).astype(BF16)

    idx16, d_cols, caps, S_t, cap_off, counts = _build_schedule(
        edge_src, edge_dst)

    p = (counts > 0).astype(np.float32)
    s2 = p / np.maximum(counts, 1).astype(np.float32)

    rhs_A, rhs_B, bias2 = _fold_weights(
        np.asarray(W_self, np.float32), np.asarray(b_self, np.float32),
        np.asarray(W_neigh, np.float32), np.asarray(b_neigh, np.float32),
        np.asarray(W_gate, np.float32), np.asarray(b_gate, np.float32))

    trivial_ln = (np.allclose(ln_gamma, 1.0) and np.allclose(ln_beta, 0.0))

    per_core = []
    for c in range(N_CORES):
        rows = slice(c * NPC, (c + 1) * NPC)
        idx_segs = []
        for g in range(N_GROUPS):
            for b in range(N_BANKS):
                idx_segs.append(_wrap_idx16(idx16[c, g, b]))
        idx_all = np.concatenate(idx_segs, axis=1)  # [128, sum(n/16)]
        hc = h_bf16[rows]  # [NPC, 128] bf16
        # block-permuted rows: DRAM row p*TPC + t holds node t*128+p, so one
        # contiguous DMA lands node t*128+p at SBUF [p, t*128:(t+1)*128]
        h_blk = np.ascontiguousarray(
            hc.reshape(TPC, 128, D).transpose(1, 0, 2).reshape(NPC, D))
        hT_blk = np.ascontiguousarray(hc.T)  # [128, NPC] bf16
        bp2 = np.stack([np.ones(NPC, np.float32), p[rows]]).astype(BF16)
        per_core.append(dict(
            h_blk=h_blk,
            hT_blk=hT_blk,
            h_gather=h_bf16,
            idx_all=idx_all,
            d_cols=np.ascontiguousarray(d_cols[c]),
            s2_cols=np.ascontiguousarray(
                s2[rows].reshape(TPC, 128).T),       # [128, TPC]
            bp2=np.ascontiguousarray(bp2),           # [2, NPC] bf16
            rhs_A=rhs_A, rhs_B=rhs_B, bias2=bias2,
            iota=np.ascontiguousarray(
                np.broadcast_to(np.arange(128, dtype=np.float32), (128, 128))
            ).astype(BF16),
        ))
    meta = dict(caps=caps, S_t=S_t, cap_off=cap_off, trivial_ln=trivial_ln,
                ln_gamma=np.asarray(ln_gamma, np.float32),
                ln_beta=np.asarray(ln_beta, np.float32))
    return per_core, meta


# ---------------------------------------------------------------------------
# Bass device kernel
# ---------------------------------------------------------------------------

_BASS_CACHE = {}


def _build_bass(caps, S_t, cap_off, trivial_ln, idxcols):
    import concourse.bacc as bacc
    import concourse.bass as bass
    import concourse.tile as tile
    from concourse import mybir

    f32 = mybir.dt.float32
    bf16 = mybir.dt.bfloat16
    fp8 = mybir.dt.float8e4
    i16 = mybir.dt.int16
    Alu = mybir.AluOpType
    Act = mybir.ActivationFunctionType

    nc = bacc.Bacc("TRN2", target_bir_lowering=False, debug=False,
                   num_devices=N_CORES)

    h_blk_d = nc.dram_tensor("h_blk", [NPC, D], bf16, kind="ExternalInput")
    hT_blk_d = nc.dram_tensor("hT_blk", [D, NPC], bf16, kind="ExternalInput")
    h_gather = nc.dram_tensor("h_gather", [N_PAD, D], bf16, kind="ExternalInput")
    idx_all = nc.dram_tensor("idx_all", [128, idxcols], i16, kind="ExternalInput")
    d_cols_d = nc.dram_tensor("d_cols", [128, TPC * S_t], f32, kind="ExternalInput")
    s2_cols_d = nc.dram_tensor("s2_cols", [128, TPC], f32, kind="ExternalInput")
    bp2_d = nc.dram_tensor("bp2", [2, NPC], bf16, kind="ExternalInput")
    rhs_A_d = nc.dram_tensor("rhs_A", [D, 2 * D], bf16, kind="ExternalInput")
    rhs_B_d = nc.dram_tensor("rhs_B", [D, 2 * D], bf16, kind="ExternalInput")
    bias2_d = nc.dram_tensor("bias2", [2, 2 * D], bf16, kind="ExternalInput")
    iota_d = nc.dram_tensor("iota", [128, 128], bf16, kind="ExternalInput")
    out_d = nc.dram_tensor("out", [NPC, D], bf16, kind="ExternalOutput")

    b_off = [TG * sum(caps[:b]) for b in range(N_BANKS)]  # G slot region start
    seg_w = [TG * caps[b] * 128 // 16 for b in range(N_BANKS)]  # idx cols/(g,b)

    with tile.TileContext(nc) as tc:
        with (
            tc.tile_pool(name="consts", bufs=1) as cpool,
            tc.tile_pool(name="gbuf", bufs=2) as gpool,
            tc.tile_pool(name="ptbuf", bufs=2) as ptpool,
            tc.tile_pool(name="vbuf", bufs=2) as vpool,
            tc.tile_pool(name="work", bufs=3) as wpool,
            tc.tile_pool(name="small", bufs=2) as spool,
            tc.tile_pool(name="psS", bufs=2, space="PSUM") as psS,
            tc.tile_pool(name="psA", bufs=2, space="PSUM") as psA,
            tc.tile_pool(name="psB", bufs=2, space="PSUM") as psB,
        ):
            # --- constants, loaded once ---
            idx_sb = cpool.tile([128, idxcols], i16)
            nc.sync.dma_start(out=idx_sb[:], in_=idx_all[:])
            d_cols_sb = cpool.tile([128, TPC * S_t], f32)
            nc.sync.dma_start(out=d_cols_sb[:], in_=d_cols_d[:])
            s2_sb = cpool.tile([128, TPC], f32)
            nc.sync.dma_start(out=s2_sb[:], in_=s2_cols_d[:])
            bp2_sb = cpool.tile([2, NPC], bf16)
            nc.sync.dma_start(out=bp2_sb[:], in_=bp2_d[:])
            rhsA_sb = cpool.tile([D, 2 * D], bf16)
            nc.sync.dma_start(out=rhsA_sb[:], in_=rhs_A_d[:])
            rhsB_sb = cpool.tile([D, 2 * D], bf16)
            nc.sync.dma_start(out=rhsB_sb[:], in_=rhs_B_d[:])
            bias2_sb = cpool.tile([2, 2 * D], bf16)
            nc.sync.dma_start(out=bias2_sb[:], in_=bias2_d[:])
            iota_sb = cpool.tile([128, 128], bf16)
            nc.sync.dma_start(out=iota_sb[:], in_=iota_d[:])
            eps_sb = cpool.tile([128, 1], f32)
            nc.vector.memset(eps_sb[:], LN_EPS)
            if not trivial_ln:
                gamma_sb = cpool.tile([128, D], f32)
                nc.gpsimd.dma_start(
                    out=gamma_sb[:],
                    in_=bass.AP(tensor=nc.dram_tensor(
                        "ln_gamma", [1, D], f32, kind="ExternalInput"),
                        offset=0, ap=[[0, 128], [1, D]]))
                beta_sb = cpool.tile([128, D], f32)
                nc.gpsimd.dma_start(
                    out=beta_sb[:],
                    in_=bass.AP(tensor=nc.dram_tensor(
                        "ln_beta", [1, D], f32, kind="ExternalInput"),
                        offset=0, ap=[[0, 128], [1, D]]))

            idx_off = [0]
            for g in range(N_GROUPS):
                for b in range(N_BANKS):
                    idx_off.append(idx_off[-1] + seg_w[b])

            for g in range(N_GROUPS):
                G = gpool.tile([128, TG * S_t, 128], bf16, tag="G")
                for b in range(N_BANKS):
                    o = idx_off[g * N_BANKS + b]
                    nidx = TG * caps[b] * 128
                    nc.gpsimd.dma_gather(
                        out_ap=G[:, b_off[b] : b_off[b] + TG * caps[b], :],
                        in_ap=h_gather[b * BANK :, :],
                        idxs_ap=idx_sb[:, o : o + seg_w[b]],
                        num_idxs=nidx,
                        num_idxs_reg=nidx,
                        elem_size=D,
                        single_packet=False,
                    )
                # group-level buffers (h / hT streamed per group)
                goff = g * TG * D
                h_g = vpool.tile([128, TG * D], bf16, tag="h_g")
                nc.sync.dma_start(
                    out=h_g[:],
                    in_=bass.AP(tensor=h_blk_d, offset=goff,
                                ap=[[TPC * D, 128], [1, TG * D]]))
                hT_g = vpool.tile([128, TG * D], bf16, tag="hT_g")
                nc.sync.dma_start(
                    out=hT_g[:],
                    in_=bass.AP(tensor=hT_blk_d, offset=goff,
                                ap=[[NPC, 128], [1, TG * D]]))
                out_g = vpool.tile([128, TG * D], bf16, tag="out_g")
                v_g = vpool.tile([128, TG * D], bf16, tag="v")
                sum_g = spool.tile([128, TG], f32, tag="sumv")
                sq_g = spool.tile([128, TG], f32, tag="sv2")
                # ---------------- phase A: gate + v ----------------
                for ti in range(TG):
                    t = g * TG + ti
                    tc0, tc1 = t * 128, (t + 1) * 128
                    # one-hot P^T for the slots
                    PT = ptpool.tile([128, S_t, 128], fp8, tag="PT")
                    for k in range(S_t):
                        col = t * S_t + k
                        nc.vector.tensor_scalar(
                            out=PT[:, k, :], in0=iota_sb[:],
                            scalar1=d_cols_sb[:, col : col + 1],
                            scalar2=None, op0=Alu.is_equal)
                    # segment sum: segT [dim, dst]
                    ps_S = psS.tile([128, 128], f32, tag="psS")
                    nmm = 0
                    for b in range(N_BANKS):
                        for j in range(caps[b]):
                            slot = b_off[b] + ti * caps[b] + j
                            k = cap_off[b] + j
                            nc.tensor.matmul(
                                ps_S[:], lhsT=G[:, slot, :], rhs=PT[:, k, :],
                                start=(nmm == 0), stop=(nmm == S_t - 1))
                            nmm += 1
                    segT = wpool.tile([128, 128], bf16, tag="segT")
                    nc.scalar.copy(out=segT[:], in_=ps_S[:])

                    # dense matmuls
                    ps_A = psA.tile([128, 2 * D], f32, tag="psA")
                    nc.tensor.matmul(ps_A[:], lhsT=hT_g[:, ti * D : (ti + 1) * D],
                                     rhs=rhsA_sb[:], start=True, stop=False)
                    nc.tensor.matmul(ps_A[:], lhsT=bp2_sb[:, tc0:tc1],
                                     rhs=bias2_sb[:], start=False, stop=True)
                    ps_B = psB.tile([128, 2 * D], f32, tag="psB")
                    nc.tensor.matmul(ps_B[:], lhsT=segT[:], rhs=rhsB_sb[:],
                                     start=True, stop=True)

                    # Y = s2 * ps_B + ps_A  (scalar applies the per-node s2
                    # scale while moving ps_B out of PSUM; DVE adds ps_A)
                    Bs2 = wpool.tile([128, 2 * D], bf16, tag="Bs2")
                    nc.scalar.mul(out=Bs2[:], in_=ps_B[:],
                                  mul=s2_sb[:, t : t + 1])
                    Y = wpool.tile([128, 2 * D], bf16, tag="Y")
                    nc.vector.tensor_tensor(out=Y[:], in0=ps_A[:], in1=Bs2[:],
                                            op=Alu.add)
                    h_t = h_g[:, ti * D : (ti + 1) * D]
                    g_sb = wpool.tile([128, D], bf16, tag="g_sb")
                    nc.scalar.activation(out=g_sb[:], in_=Y[:, D:],
                                         func=Act.Sigmoid)
                    t1 = wpool.tile([128, D], bf16, tag="t1")
                    nc.vector.tensor_tensor(out=t1[:], in0=Y[:, :D], in1=h_t,
                                            op=Alu.subtract)
                    t2 = wpool.tile([128, D], bf16, tag="t2")
                    nc.vector.tensor_tensor(out=t2[:], in0=g_sb[:], in1=t1[:],
                                            op=Alu.mult)
                    v_sl = v_g[:, ti * D : (ti + 1) * D]
                    nc.vector.scalar_tensor_tensor(
                        out=v_sl, in0=t2[:], scalar=0.0, in1=h_t,
                        op0=Alu.add, op1=Alu.add,
                        accum_out=sum_g[:, ti : ti + 1])
                    vsq = wpool.tile([128, D], bf16, tag="vsq")
                    nc.scalar.activation(out=vsq[:], in_=v_sl,
                                         func=Act.Square,
                                         accum_out=sq_g[:, ti : ti + 1])
                # ---------------- phase B: layernorm (batched stats) -------
                mu_g = spool.tile([128, TG], f32, tag="mu")
                nc.vector.tensor_scalar(out=mu_g[:], in0=sum_g[:],
                                        scalar1=1.0 / D, scalar2=None,
                                        op0=Alu.mult)
                msq_g = spool.tile([128, TG], f32, tag="msq")
                nc.vector.tensor_scalar(out=msq_g[:], in0=sq_g[:],
                                        scalar1=1.0 / D, scalar2=None,
                                        op0=Alu.mult)
                musq_g = spool.tile([128, TG], f32, tag="musq")
                nc.vector.tensor_tensor(out=musq_g[:], in0=mu_g[:],
                                        in1=mu_g[:], op=Alu.mult)
                var_g = spool.tile([128, TG], f32, tag="var")
                nc.vector.tensor_tensor(out=var_g[:], in0=msq_g[:],
                                        in1=musq_g[:], op=Alu.subtract)
                std_g = spool.tile([128, TG], f32, tag="std")
                nc.scalar.activation(out=std_g[:], in_=var_g[:], func=Act.Sqrt,
                                     bias=eps_sb[:])
                rstd_g = spool.tile([128, TG], f32, tag="rstd")
                nc.vector.reciprocal(out=rstd_g[:], in_=std_g[:])
                bl_g = spool.tile([128, TG], f32, tag="bl")
                nc.vector.scalar_tensor_tensor(
                    out=bl_g[:], in0=mu_g[:], scalar=-1.0, in1=rstd_g[:],
                    op0=Alu.mult, op1=Alu.mult)
                for ti in range(TG):
                    t = g * TG + ti
                    tc0, tc1 = t * 128, (t + 1) * 128
                    v_sl = v_g[:, ti * D : (ti + 1) * D]
                    if trivial_ln:
                        nc.scalar.activation(out=out_g[:, ti * D : (ti + 1) * D],
                                             in_=v_sl, func=Act.Relu,
                                             bias=bl_g[:, ti : ti + 1],
                                             scale=rstd_g[:, ti : ti + 1])
                    else:
                        z = wpool.tile([128, D], f32, tag="z")
                        nc.scalar.activation(out=z[:], in_=v_sl,
                                             func=Act.Identity,
                                             bias=bl_g[:, ti : ti + 1],
                                             scale=rstd_g[:, ti : ti + 1])
                        nc.vector.tensor_tensor(out=z[:], in0=z[:],
                                                in1=gamma_sb[:], op=Alu.mult)
                        nc.vector.tensor_tensor(out=z[:], in0=z[:],
                                                in1=beta_sb[:], op=Alu.add)
                        nc.scalar.activation(out=out_g[:, ti * D : (ti + 1) * D],
                                             in_=z[:], func=Act.Relu)
                # store this group's output slice (host un-permutes)
                nc.sync.dma_start(
                    out=bass.AP(tensor=out_d, offset=goff,
                                ap=[[TPC * D, 128], [1, TG * D]]),
                    in_=out_g[:])
    nc.compile()
    return nc


def kernel(**inputs):
    from concourse.bass_utils import run_bass_kernel_spmd

    per_core, meta = _prep(**{k: np.asarray(v) for k, v in inputs.items()})
    idxcols = per_core[0]["idx_all"].shape[1]
    key = (tuple(meta["caps"]), meta["trivial_ln"], idxcols)
    if key not in _BASS_CACHE:
        _BASS_CACHE[key] = _build_bass(
            meta["caps"], meta["S_t"], meta["cap_off"], meta["trivial_ln"],
            idxcols)
    nc = _BASS_CACHE[key]

    in_maps = []
    for pc in per_core:
        m = dict(pc)
        if not meta["trivial_ln"]:
            m["ln_gamma"] = meta["ln_gamma"][None]
            m["ln_beta"] = meta["ln_beta"][None]
        in_maps.append(m)
    res = run_bass_kernel_spmd(nc, in_maps, core_ids=list(range(N_CORES)))
    outs = []
    for c in range(N_CORES):
        o = np.asarray(res.results[c]["out"])  # [NPC, D] bf16, block-permuted
        o = o.reshape(128, TPC, D).transpose(1, 0, 2).reshape(NPC, D)
        outs.append(o.astype(np.float32))
    return np.concatenate(outs, 0)[:N]


# revision 15
# speedup vs baseline: 1.1524x; 1.1433x over previous
"""GNN message-passing layer (DAGLayer) on 8 Trainium2 NeuronCores.

Strategy:
  - Shard destination nodes 8 ways (12544 nodes = 98 tiles of 128 per core,
    N padded 100000 -> 100352). Replicate h (as a bf16 gather table) so
    cross-partition edges need no communication.
  - Segment-sum over edges via TensorE matmuls: for each 128-dst tile,
    gather h[src] rows (dma_gather, bf16) for its edges in chunks of 128,
    build one-hot P^T on DVE (single-op is_equal vs iota, fp8 out),
    accumulate segT = G^T @ P^T in PSUM ([dim, dst] layout -> lhsT reuse).
  - dma_gather indices are int16, so the gather table is split into 4
    row-banks (< 32768 rows each). Nodes are assigned to banks by a greedy
    balancer (heaviest out-degree first) so that every (dst-tile, bank)
    edge count fits slot caps [2,2,2,1] -- 7 chunk slots of 128 per tile
    instead of the naive 8, cutting gather traffic, one-hot builds and
    segment matmuls by 12.5%. The slot grid is data-independent, so the
    compiled program is identical on all cores (SPMD); only the data
    (indices, one-hot columns) differs. Caps are recomputed from the data
    at runtime, so an unlucky edge distribution just falls back to wider
    caps with the same code.
  - Dense path per tile with host-folded weights, all-bf16 matmuls:
      ps_A = h @ [Ws^T | (Wg1+Wg2@Ws)^T]
           + [1|p] @ [[b_s|b_g+Wg2@b_s]; [b_n|Wg2@b_n]]   (K=2 matmul)
      ps_B = seg @ [Wn^T | (Wg2@Wn)^T]
      Y    = s2 * ps_B + ps_A   (scalar engine scales ps_B out of PSUM
                                 with the per-node s2; DVE adds ps_A)
      m, gp = Y[:, :128], Y[:, 128:]
      v = h + sigmoid(gp) * (m - h)      (bf16 elementwise, f32 accums)
      out = relu((v - mu) * rstd)        [bf16, cast to f32 on host]
  - Per-group two-phase scalar schedule to avoid ACT_TABLE_LOAD thrash:
    phase A (gate) uses {Sigmoid, Square, Copy} = sigmoid table; v is
    buffered per group; phase B (layernorm) uses {Sqrt, Relu} = sqrt
    table; LN statistics are computed batched [128, TG] per group.
  - h / hT / out are streamed per group as single large DMAs (the host
    permutes h rows so one contiguous block DMA lands node t*128+p on
    SBUF partition p); output is stored per group the same way.
"""

import numpy as np

try:
    import ml_dtypes

    BF16 = np.dtype(ml_dtypes.bfloat16)
except ImportError:  # pragma: no cover
    BF16 = None

N = 100000
E = 600000
D = 128
N_CORES = 8
NPC = 12544            # nodes per core
TPC = NPC // 128       # 98 tiles per core
N_PAD = NPC * N_CORES  # 100352
N_BANKS = 4
BANK = N_PAD // N_BANKS  # 25088
TG = 14                # tiles per gather group
N_GROUPS = TPC // TG   # 7
LN_EPS = 1e-5


def _assign_banks(edge_src, edge_dst):
    """Greedy node->bank assignment for the gather table so that per
    (dst-tile, bank) edge counts fit caps [2,2,2,1] (<=256,256,256,128).
    The table layout is free; only each bank must stay under 32767 rows
    (int16 gather indices). Returns (pos[node] -> table row, bases[5])."""
    ROWCAP = 32767
    n_tiles = N_PAD // 128
    deg = np.bincount(edge_src, minlength=N_PAD)
    tiles = (edge_dst // 128).astype(np.int64)
    order = np.argsort(edge_src, kind="stable")
    tl = tiles[order].tolist()
    starts = np.searchsorted(edge_src[order], np.arange(N_PAD)).tolist()
    starts.append(len(tl))
    LIM = (252.0, 252.0, 252.0, 124.0)
    cnt = [[0] * n_tiles for _ in range(N_BANKS)]
    sizes = [0] * N_BANKS
    bank_of = np.full(N_PAD, -1, np.int8)
    for u in np.argsort(-deg, kind="stable").tolist():
        s, e = starts[u], starts[u + 1]
        if s == e:
            break  # remaining nodes have no out-edges
        d = {}
        for i in range(s, e):
            t = tl[i]
            d[t] = d.get(t, 0) + 1
        best_b, best_sc = -1, None
        for b in range(N_BANKS):
            if sizes[b] >= ROWCAP:
                continue
            cb, lim = cnt[b], LIM[b]
            sc = max((cb[t] + m) / lim for t, m in d.items())
            if best_sc is None or sc < best_sc:
                best_b, best_sc = b, sc
        cb = cnt[best_b]
        for t, m in d.items():
            cb[t] += m
        sizes[best_b] += 1
        bank_of[u] = best_b
    # nodes with no out-edges: fill remaining row capacity
    rem = np.flatnonzero(bank_of < 0)
    off = 0
    for b in range(N_BANKS):
        take = min(ROWCAP - sizes[b], len(rem) - off)
        if take > 0:
            bank_of[rem[off : off + take]] = b
            sizes[b] += take
            off += take
    assert off == len(rem)
    bases = np.concatenate([[0], np.cumsum(sizes)])
    # pos: stable order within each bank
    pos = np.zeros(N_PAD, np.int64)
    for b in range(N_BANKS):
        nodes_b = np.flatnonzero(bank_of == b)
        pos[nodes_b] = bases[b] + np.arange(len(nodes_b))
    return pos, bases.astype(np.int64)


def _build_schedule(edge_src, edge_dst, pos, bases):
    """Chunk/slot schedule. Returns per-core idx16 + d_cols arrays and the
    (core-independent) slot capacities."""
    edge_src = np.asarray(edge_src, np.int64)
    edge_dst = np.asarray(edge_dst, np.int64)

    counts = np.bincount(edge_dst, minlength=N_PAD)

    tile_g = edge_dst // 128          # global dst tile 0..783
    pos_e = pos[edge_src]             # table row per edge
    bank = np.searchsorted(bases[1:], pos_e, side="right")  # 0..3
    key = tile_g * N_BANKS + bank
    order = np.argsort(key, kind="stable")
    src_s = pos_e[order]              # table rows, sorted by (tile, bank)
    dst_s = edge_dst[order]

    n_tiles_g = N_PAD // 128          # 784
    cnt = np.bincount(key[order], minlength=n_tiles_g * N_BANKS).reshape(
        n_tiles_g, N_BANKS
    )
    starts = np.zeros((n_tiles_g, N_BANKS), np.int64)
    starts.reshape(-1)[1:] = np.cumsum(cnt.reshape(-1))[:-1]

    caps = [max(1, int(np.ceil(cnt[:, b].max() / 128.0))) for b in range(N_BANKS)]
    S_t = sum(caps)                   # slots per tile
    cap_off = np.cumsum([0] + caps)   # slot offset of bank b within a tile

    # per-core arrays
    idx16 = np.zeros((N_CORES, N_GROUPS, N_BANKS), object)
    d_cols = np.full((N_CORES, 128, TPC * S_t), -1.0, np.float32)

    for c in range(N_CORES):
        for g in range(N_GROUPS):
            for b in range(N_BANKS):
                lst = np.zeros(TG * caps[b] * 128, np.int16)
                for ti in range(TG):
                    t_local = g * TG + ti
                    t_glob = c * TPC + t_local
                    n_e = int(cnt[t_glob, b])
                    s0 = int(starts[t_glob, b])
                    srcs = src_s[s0 : s0 + n_e] - bases[b]
                    dsts = dst_s[s0 : s0 + n_e] - t_glob * 128
                    base = ti * caps[b] * 128
                    lst[base : base + n_e] = srcs.astype(np.int16)
                    for j in range(caps[b]):
                        col = t_local * S_t + cap_off[b] + j
                        seg = dsts[j * 128 : (j + 1) * 128]
                        d_cols[c, : len(seg), col] = seg
                idx16[c, g, b] = lst
    return idx16, d_cols, caps, S_t, cap_off, counts


def _wrap_idx16(flat):
    """dma_gather index layout: idx i -> [i % 16, i // 16], replicated to
    all 128 partitions."""
    n = len(flat)
    w = np.zeros((128, n // 16), np.int16)
    w16 = flat.reshape(n // 16, 16).T  # [16, n/16]
    w[:16] = w16
    w[16:] = np.tile(w16, (7, 1))
    return w


def _fold_weights(W_self, b_self, W_neigh, b_neigh, W_gate, b_gate):
    Wg1 = W_gate[:, :D]
    Wg2 = W_gate[:, D:]
    rhs_A = np.concatenate([W_self.T, (Wg1 + Wg2 @ W_self).T], 1)  # [128,256]
    rhs_B = np.concatenate([W_neigh.T, (Wg2 @ W_neigh).T], 1)      # [128,256]
    bias_A = np.concatenate([b_self, b_gate + Wg2 @ b_self])        # [256]
    bias_Bp = np.concatenate([b_neigh, Wg2 @ b_neigh])              # [256]
    bias2 = np.stack([bias_A, bias_Bp])                             # [2,256]
    return (
        np.ascontiguousarray(rhs_A).astype(BF16),
        np.ascontiguousarray(rhs_B).astype(BF16),
        np.ascontiguousarray(bias2).astype(BF16),
    )


def _prep(h, edge_src, edge_dst, W_self, b_self, W_neigh, b_neigh, W_gate,
          b_gate, ln_gamma, ln_beta):
    h = np.asarray(h, np.float32)
    h_pad = np.zeros((N_PAD, D), np.float32)
    h_pad[:N] = h
    h_bf16 = np.ascontiguousarray(h_pad).astype(BF16)

    pos, bases = _assign_banks(np.asarray(edge_src, np.int64),
                               np.asarray(edge_dst, np.int64))
    idx16, d_cols, caps, S_t, cap_off, counts = _build_schedule(
        edge_src, edge_dst, pos, bases)
    row_order = np.argsort(pos)       # table row -> node
    h_tab = np.ascontiguousarray(h_bf16[row_order])

    p = (counts > 0).astype(np.float32)
    s2 = p / np.maximum(counts, 1).astype(np.float32)

    rhs_A, rhs_B, bias2 = _fold_weights(
        np.asarray(W_self, np.float32), np.asarray(b_self, np.float32),
        np.asarray(W_neigh, np.float32), np.asarray(b_neigh, np.float32),
        np.asarray(W_gate, np.float32), np.asarray(b_gate, np.float32))

    trivial_ln = (np.allclose(ln_gamma, 1.0) and np.allclose(ln_beta, 0.0))

    per_core = []
    for c in range(N_CORES):
        rows = slice(c * NPC, (c + 1) * NPC)
        idx_segs = []
        for g in range(N_GROUPS):
            for b in range(N_BANKS):
                idx_segs.append(_wrap_idx16(idx16[c, g, b]))
        idx_all = np.concatenate(idx_segs, axis=1)  # [128, sum(n/16)]
        hc = h_bf16[rows]  # [NPC, 128] bf16
        # block-permuted rows: DRAM row p*TPC + t holds node t*128+p, so one
        # contiguous DMA lands node t*128+p at SBUF [p, t*128:(t+1)*128]
        h_blk = np.ascontiguousarray(
            hc.reshape(TPC, 128, D).transpose(1, 0, 2).reshape(NPC, D))
        hT_blk = np.ascontiguousarray(hc.T)  # [128, NPC] bf16
        bp2 = np.stack([np.ones(NPC, np.float32), p[rows]]).astype(BF16)
        per_core.append(dict(
            h_blk=h_blk,
            hT_blk=hT_blk,
            h_gather=h_tab,
            idx_all=idx_all,
            d_cols=np.ascontiguousarray(d_cols[c]),
            s2_cols=np.ascontiguousarray(
                s2[rows].reshape(TPC, 128).T),       # [128, TPC]
            bp2=np.ascontiguousarray(bp2),           # [2, NPC] bf16
            rhs_A=rhs_A, rhs_B=rhs_B, bias2=bias2,
            iota=np.ascontiguousarray(
                np.broadcast_to(np.arange(128, dtype=np.float32), (128, 128))
            ).astype(BF16),
        ))
    meta = dict(caps=caps, S_t=S_t, cap_off=cap_off, trivial_ln=trivial_ln,
                bases=tuple(int(x) for x in bases[:N_BANKS]),
                ln_gamma=np.asarray(ln_gamma, np.float32),
                ln_beta=np.asarray(ln_beta, np.float32))
    return per_core, meta


# ---------------------------------------------------------------------------
# Bass device kernel
# ---------------------------------------------------------------------------

_BASS_CACHE = {}


def _build_bass(caps, S_t, cap_off, trivial_ln, idxcols, bases):
    import concourse.bacc as bacc
    import concourse.bass as bass
    import concourse.tile as tile
    from concourse import mybir

    f32 = mybir.dt.float32
    bf16 = mybir.dt.bfloat16
    fp8 = mybir.dt.float8e4
    i16 = mybir.dt.int16
    Alu = mybir.AluOpType
    Act = mybir.ActivationFunctionType

    nc = bacc.Bacc("TRN2", target_bir_lowering=False, debug=False,
                   num_devices=N_CORES)

    h_blk_d = nc.dram_tensor("h_blk", [NPC, D], bf16, kind="ExternalInput")
    hT_blk_d = nc.dram_tensor("hT_blk", [D, NPC], bf16, kind="ExternalInput")
    h_gather = nc.dram_tensor("h_gather", [N_PAD, D], bf16, kind="ExternalInput")
    idx_all = nc.dram_tensor("idx_all", [128, idxcols], i16, kind="ExternalInput")
    d_cols_d = nc.dram_tensor("d_cols", [128, TPC * S_t], f32, kind="ExternalInput")
    s2_cols_d = nc.dram_tensor("s2_cols", [128, TPC], f32, kind="ExternalInput")
    bp2_d = nc.dram_tensor("bp2", [2, NPC], bf16, kind="ExternalInput")
    rhs_A_d = nc.dram_tensor("rhs_A", [D, 2 * D], bf16, kind="ExternalInput")
    rhs_B_d = nc.dram_tensor("rhs_B", [D, 2 * D], bf16, kind="ExternalInput")
    bias2_d = nc.dram_tensor("bias2", [2, 2 * D], bf16, kind="ExternalInput")
    iota_d = nc.dram_tensor("iota", [128, 128], bf16, kind="ExternalInput")
    out_d = nc.dram_tensor("out", [NPC, D], bf16, kind="ExternalOutput")

    b_off = [TG * sum(caps[:b]) for b in range(N_BANKS)]  # G slot region start
    seg_w = [TG * caps[b] * 128 // 16 for b in range(N_BANKS)]  # idx cols/(g,b)

    with tile.TileContext(nc) as tc:
        with (
            tc.tile_pool(name="consts", bufs=1) as cpool,
            tc.tile_pool(name="gbuf", bufs=2) as gpool,
            tc.tile_pool(name="ptbuf", bufs=2) as ptpool,
            tc.tile_pool(name="vbuf", bufs=2) as vpool,
            tc.tile_pool(name="work", bufs=3) as wpool,
            tc.tile_pool(name="small", bufs=2) as spool,
            tc.tile_pool(name="psS", bufs=2, space="PSUM") as psS,
            tc.tile_pool(name="psA", bufs=2, space="PSUM") as psA,
            tc.tile_pool(name="psB", bufs=2, space="PSUM") as psB,
        ):
            # --- constants, loaded once ---
            idx_sb = cpool.tile([128, idxcols], i16)
            nc.sync.dma_start(out=idx_sb[:], in_=idx_all[:])
            d_cols_sb = cpool.tile([128, TPC * S_t], f32)
            nc.sync.dma_start(out=d_cols_sb[:], in_=d_cols_d[:])
            s2_sb = cpool.tile([128, TPC], f32)
            nc.sync.dma_start(out=s2_sb[:], in_=s2_cols_d[:])
            bp2_sb = cpool.tile([2, NPC], bf16)
            nc.sync.dma_start(out=bp2_sb[:], in_=bp2_d[:])
            rhsA_sb = cpool.tile([D, 2 * D], bf16)
            nc.sync.dma_start(out=rhsA_sb[:], in_=rhs_A_d[:])
            rhsB_sb = cpool.tile([D, 2 * D], bf16)
            nc.sync.dma_start(out=rhsB_sb[:], in_=rhs_B_d[:])
            bias2_sb = cpool.tile([2, 2 * D], bf16)
            nc.sync.dma_start(out=bias2_sb[:], in_=bias2_d[:])
            iota_sb = cpool.tile([128, 128], bf16)
            nc.sync.dma_start(out=iota_sb[:], in_=iota_d[:])
            eps_sb = cpool.tile([128, 1], f32)
            nc.vector.memset(eps_sb[:], LN_EPS)
            if not trivial_ln:
                gamma_sb = cpool.tile([128, D], f32)
                nc.gpsimd.dma_start(
                    out=gamma_sb[:],
                    in_=bass.AP(tensor=nc.dram_tensor(
                        "ln_gamma", [1, D], f32, kind="ExternalInput"),
                        offset=0, ap=[[0, 128], [1, D]]))
                beta_sb = cpool.tile([128, D], f32)
                nc.gpsimd.dma_start(
                    out=beta_sb[:],
                    in_=bass.AP(tensor=nc.dram_tensor(
                        "ln_beta", [1, D], f32, kind="ExternalInput"),
                        offset=0, ap=[[0, 128], [1, D]]))

            idx_off = [0]
            for g in range(N_GROUPS):
                for b in range(N_BANKS):
                    idx_off.append(idx_off[-1] + seg_w[b])

            for g in range(N_GROUPS):
                G = gpool.tile([128, TG * S_t, 128], bf16, tag="G")
                for b in range(N_BANKS):
                    o = idx_off[g * N_BANKS + b]
                    nidx = TG * caps[b] * 128
                    nc.gpsimd.dma_gather(
                        out_ap=G[:, b_off[b] : b_off[b] + TG * caps[b], :],
                        in_ap=h_gather[bases[b] :, :],
                        idxs_ap=idx_sb[:, o : o + seg_w[b]],
                        num_idxs=nidx,
                        num_idxs_reg=nidx,
                        elem_size=D,
                        single_packet=False,
                    )
                # group-level buffers (h / hT streamed per group)
                goff = g * TG * D
                h_g = vpool.tile([128, TG * D], bf16, tag="h_g")
                nc.sync.dma_start(
                    out=h_g[:],
                    in_=bass.AP(tensor=h_blk_d, offset=goff,
                                ap=[[TPC * D, 128], [1, TG * D]]))
                hT_g = vpool.tile([128, TG * D], bf16, tag="hT_g")
                nc.sync.dma_start(
                    out=hT_g[:],
                    in_=bass.AP(tensor=hT_blk_d, offset=goff,
                                ap=[[NPC, 128], [1, TG * D]]))
                out_g = vpool.tile([128, TG * D], bf16, tag="out_g")
                v_g = vpool.tile([128, TG * D], bf16, tag="v")
                sum_g = spool.tile([128, TG], f32, tag="sumv")
                sq_g = spool.tile([128, TG], f32, tag="sv2")
                # ---------------- phase A: gate + v ----------------
                for ti in range(TG):
                    t = g * TG + ti
                    tc0, tc1 = t * 128, (t + 1) * 128
                    # one-hot P^T for the slots
                    PT = ptpool.tile([128, S_t, 128], fp8, tag="PT")
                    for k in range(S_t):
                        col = t * S_t + k
                        nc.vector.tensor_scalar(
                            out=PT[:, k, :], in0=iota_sb[:],
                            scalar1=d_cols_sb[:, col : col + 1],
                            scalar2=None, op0=Alu.is_equal)
                    # segment sum: segT [dim, dst]
                    ps_S = psS.tile([128, 128], f32, tag="psS")
                    nmm = 0
                    for b in range(N_BANKS):
                        for j in range(caps[b]):
                            slot = b_off[b] + ti * caps[b] + j
                            k = cap_off[b] + j
                            nc.tensor.matmul(
                                ps_S[:], lhsT=G[:, slot, :], rhs=PT[:, k, :],
                                start=(nmm == 0), stop=(nmm == S_t - 1))
                            nmm += 1
                    segT = wpool.tile([128, 128], bf16, tag="segT")
                    nc.scalar.copy(out=segT[:], in_=ps_S[:])

                    # dense matmuls
                    ps_A = psA.tile([128, 2 * D], f32, tag="psA")
                    nc.tensor.matmul(ps_A[:], lhsT=hT_g[:, ti * D : (ti + 1) * D],
                                     rhs=rhsA_sb[:], start=True, stop=False)
                    nc.tensor.matmul(ps_A[:], lhsT=bp2_sb[:, tc0:tc1],
                                     rhs=bias2_sb[:], start=False, stop=True)
                    ps_B = psB.tile([128, 2 * D], f32, tag="psB")
                    nc.tensor.matmul(ps_B[:], lhsT=segT[:], rhs=rhsB_sb[:],
                                     start=True, stop=True)

                    # Y = s2 * ps_B + ps_A  (scalar applies the per-node s2
                    # scale while moving ps_B out of PSUM; DVE adds ps_A)
                    Bs2 = wpool.tile([128, 2 * D], bf16, tag="Bs2")
                    nc.scalar.mul(out=Bs2[:], in_=ps_B[:],
                                  mul=s2_sb[:, t : t + 1])
                    Y = wpool.tile([128, 2 * D], bf16, tag="Y")
                    nc.vector.tensor_tensor(out=Y[:], in0=ps_A[:], in1=Bs2[:],
                                            op=Alu.add)
                    h_t = h_g[:, ti * D : (ti + 1) * D]
                    g_sb = wpool.tile([128, D], bf16, tag="g_sb")
                    nc.scalar.activation(out=g_sb[:], in_=Y[:, D:],
                                         func=Act.Sigmoid)
                    t1 = wpool.tile([128, D], bf16, tag="t1")
                    nc.vector.tensor_tensor(out=t1[:], in0=Y[:, :D], in1=h_t,
                                            op=Alu.subtract)
                    t2 = wpool.tile([128, D], bf16, tag="t2")
                    nc.vector.tensor_tensor(out=t2[:], in0=g_sb[:], in1=t1[:],
                                            op=Alu.mult)
                    v_sl = v_g[:, ti * D : (ti + 1) * D]
                    nc.vector.scalar_tensor_tensor(
                        out=v_sl, in0=t2[:], scalar=0.0, in1=h_t,
                        op0=Alu.add, op1=Alu.add,
                        accum_out=sum_g[:, ti : ti + 1])
                    vsq = wpool.tile([128, D], bf16, tag="vsq")
                    nc.scalar.activation(out=vsq[:], in_=v_sl,
                                         func=Act.Square,
                                         accum_out=sq_g[:, ti : ti + 1])
                # ---------------- phase B: layernorm (batched stats) -------
                mu_g = spool.tile([128, TG], f32, tag="mu")
                nc.vector.tensor_scalar(out=mu_g[:], in0=sum_g[:],
                                        scalar1=1.0 / D, scalar2=None,
                                        op0=Alu.mult)
                msq_g = spool.tile([128, TG], f32, tag="msq")
                nc.vector.tensor_scalar(out=msq_g[:], in0=sq_g[:],
                                        scalar1=1.0 / D, scalar2=None,
                                        op0=Alu.mult)
                musq_g = spool.tile([128, TG], f32, tag="musq")
                nc.vector.tensor_tensor(out=musq_g[:], in0=mu_g[:],
                                        in1=mu_g[:], op=Alu.mult)
                var_g = spool.tile([128, TG], f32, tag="var")
                nc.vector.tensor_tensor(out=var_g[:], in0=msq_g[:],
                                        in1=musq_g[:], op=Alu.subtract)
                std_g = spool.tile([128, TG], f32, tag="std")
                nc.scalar.activation(out=std_g[:], in_=var_g[:], func=Act.Sqrt,
                                     bias=eps_sb[:])
                rstd_g = spool.tile([128, TG], f32, tag="rstd")
                nc.vector.reciprocal(out=rstd_g[:], in_=std_g[:])
                bl_g = spool.tile([128, TG], f32, tag="bl")
                nc.vector.scalar_tensor_tensor(
                    out=bl_g[:], in0=mu_g[:], scalar=-1.0, in1=rstd_g[:],
                    op0=Alu.mult, op1=Alu.mult)
                for ti in range(TG):
                    t = g * TG + ti
                    tc0, tc1 = t * 128, (t + 1) * 128
                    v_sl = v_g[:, ti * D : (ti + 1) * D]
                    if trivial_ln:
                        nc.scalar.activation(out=out_g[:, ti * D : (ti + 1) * D],
                                             in_=v_sl, func=Act.Relu,
                                             bias=bl_g[:, ti : ti + 1],
                                             scale=rstd_g[:, ti : ti + 1])
                    else:
                        z = wpool.tile([128, D], f32, tag="z")
                        nc.scalar.activation(out=z[:], in_=v_sl,
                                             func=Act.Identity,
                                             bias=bl_g[:, ti : ti + 1],
                                             scale=rstd_g[:, ti : ti + 1])
                        nc.vector.tensor_tensor(out=z[:], in0=z[:],
                                                in1=gamma_sb[:], op=Alu.mult)
                        nc.vector.tensor_tensor(out=z[:], in0=z[:],
                                                in1=beta_sb[:], op=Alu.add)
                        nc.scalar.activation(out=out_g[:, ti * D : (ti + 1) * D],
                                             in_=z[:], func=Act.Relu)
                # store this group's output slice (host un-permutes)
                nc.sync.dma_start(
                    out=bass.AP(tensor=out_d, offset=goff,
                                ap=[[TPC * D, 128], [1, TG * D]]),
                    in_=out_g[:])
    nc.compile()
    return nc


def kernel(**inputs):
    from concourse.bass_utils import run_bass_kernel_spmd

    per_core, meta = _prep(**{k: np.asarray(v) for k, v in inputs.items()})
    idxcols = per_core[0]["idx_all"].shape[1]
    key = (tuple(meta["caps"]), meta["trivial_ln"], idxcols, meta["bases"])
    if key not in _BASS_CACHE:
        _BASS_CACHE[key] = _build_bass(
            meta["caps"], meta["S_t"], meta["cap_off"], meta["trivial_ln"],
            idxcols, meta["bases"])
    nc = _BASS_CACHE[key]

    in_maps = []
    for pc in per_core:
        m = dict(pc)
        if not meta["trivial_ln"]:
            m["ln_gamma"] = meta["ln_gamma"][None]
            m["ln_beta"] = meta["ln_beta"][None]
        in_maps.append(m)
    res = run_bass_kernel_spmd(nc, in_maps, core_ids=list(range(N_CORES)))
    outs = []
    for c in range(N_CORES):
        o = np.asarray(res.results[c]["out"])  # [NPC, D] bf16, block-permuted
        o = o.reshape(128, TPC, D).transpose(1, 0, 2).reshape(NPC, D)
        outs.append(o.astype(np.float32))
    return np.concatenate(outs, 0)[:N]
